# revision 9
# baseline (speedup 1.0000x reference)
"""Trainium2 Bass kernel for nn_BiSRConv2d_Down.

Reference semantics (forward values):
  out  = avgpool2x2(x)                                  [B, C, H/2, W/2]
  for branch b in {1, 2}:
    xb   = sign(out * mvk_b + mvb_b)                    (tanh STE terms cancel)
    bw   = mean|W_b|_(i,kh,kw) * sign(W_b)              per-output-channel scale
    conv = conv2d(xb, bw, pad=1)
    y_b  = out + (prelu(conv + pb0_b; alpha_b) + pb1_b)
  return concat([y1, y2], channel axis)

Strategy: data-parallel over batch on 8 cores (2 images/core).
 - avg-pool row pairs are summed inside the DMA datapath (SWDGE accum_op=add)
   so only the column-pair add runs on a compute engine; the 0.25 scale is
   folded into the sign activation's scale and the residual op.
 - conv = 9 shifted 128x128 matmuls (channels on partitions), chunk-major
   (9 taps back-to-back into one PSUM bank), branches interleaved per chunk.
   sign activations are exact in bf16 so matmuls accumulate exact integer
   sums; the per-output-channel scale rides the PSUM eviction (ScalarE
   activation with per-partition scale/bias).
 - PReLU uses prelu(v) = max(v, alpha*v) (valid for alpha in [0,1]; checked
   on host), computed in fp16 (conv term is ~2% of output
   magnitude and values are small, so fp16 rounding is ~1e-5 relative to the
   output scale; the residual path stays fp32). Final add lands in the fp32 residual tile.
"""

import os

import numpy as np

import concourse.bacc as bacc
import concourse.mybir as mybir
import concourse.tile as tile
from concourse.bass_utils import run_bass_kernel_spmd

F32 = mybir.dt.float32
BF16 = mybir.dt.bfloat16
FP16 = mybir.dt.float16
AF = mybir.ActivationFunctionType
ALU = mybir.AluOpType

B, C, H, W = 16, 128, 128, 128
NCORES = 8
IPC = B // NCORES          # images per core
HP, WP = H // 2, W // 2    # pooled height/width: 64, 64
RL = WP + 2                # padded row stride 66
NPADF = (HP + 2) * RL      # padded image size 4356
XBPLEN = NPADF + 2         # +2 tail pad so tap reads stay in-bounds
NROWCH = 7                 # output rows per PSUM chunk (7*66=462 <= 512)
# (q0, nrows, ncols) per PSUM chunk; the tail chunk is the last single row
CHUNKS = [(ci * NROWCH * RL, NROWCH, NROWCH * RL) for ci in range(9)]
CHUNKS.append((63 * RL, 1, RL))
# epilogue halves: output row ranges
HALVES = [(0, 35), (35, 29)]
POOLCH = 16                # pooled rows per input chunk (32 x rows, ~2.1 MiB)
NCH = HP // POOLCH         # input/pool/sign chunks per image


def build_nc():
    nc = bacc.Bacc(
        "TRN2", target_bir_lowering=False, debug=False, num_devices=NCORES
    )
    x_d = nc.dram_tensor("x", [IPC, C, H, W], F32, kind="ExternalInput")
    # wt: host-relaid weights, wt[b][i, t*128+o] = W_b[o, i, kh, kw], t=kh*3+kw
    wt_d = nc.dram_tensor("wt", [2, C, 9 * C], F32, kind="ExternalInput")
    # wn: natural weights flattened per output channel (for mean|W|)
    wn_d = nc.dram_tensor("wn", [2, C, 9 * C], F32, kind="ExternalInput")
    # pp: per-channel params, col 5*b+{0:mvk,1:mvb,2:pb0,3:alpha,4:pb1}
    pp_d = nc.dram_tensor("pp", [C, 10], F32, kind="ExternalInput")
    out_d = nc.dram_tensor("out", [IPC, 2 * C, HP, WP], F32, kind="ExternalOutput")

    with tile.TileContext(nc) as tc:
        with (
            tc.tile_pool(name="const", bufs=1) as cpool,
            tc.tile_pool(name="wload", bufs=1) as wpool,
            tc.tile_pool(name="xin", bufs=3) as xpool,
            tc.tile_pool(name="oasm", bufs=2) as opool,
            tc.tile_pool(name="resp", bufs=2) as respool,
            tc.tile_pool(name="ps", bufs=6, space="PSUM") as pspool,
        ):
            # ---------- weight / param prep (scalar HWDGE queue, so the
            # sync queue starts streaming x immediately) ----------
            pp_t = cpool.tile([C, 10], F32, name="pp_t")
            nc.scalar.dma_start(pp_t[:], pp_d[:])

            sgn, scale_w, sc_sign = [], [], []
            for b in range(2):
                wld = wpool.tile([C, 9 * C], F32, tag="wload", name=f"wld{b}")
                nc.scalar.dma_start(wld[:], wt_d[b])
                s = cpool.tile([C, 9 * C], BF16, name=f"sgnw{b}")
                nc.scalar.sign(s[:], wld[:])
                sgn.append(s)

                wnl = wpool.tile([C, 9 * C], F32, tag="wload", name=f"wnl{b}")
                nc.scalar.dma_start(wnl[:], wn_d[b])
                asum = cpool.tile([C, 1], F32, name=f"asum{b}")
                nc.scalar.activation(wnl[:], wnl[:], AF.Abs, accum_out=asum[:])
                sw = cpool.tile([C, 1], F32, name=f"scale_w{b}")
                nc.vector.tensor_scalar_mul(sw[:], asum[:], 1.0 / (9 * C))
                scale_w.append(sw)

                ss = cpool.tile([C, 1], F32, name=f"sc_sign{b}")
                nc.vector.tensor_scalar_mul(
                    ss[:], pp_t[:, 5 * b + 0 : 5 * b + 1], 0.25
                )
                sc_sign.append(ss)

            # padded sign-activation buffers: only the BORDERS need zeroing
            # (row 0, row 65, cols 0/65 of each row, 2-elem tail); interiors
            # are fully rewritten per image.
            xbp = [
                [cpool.tile([C, XBPLEN], BF16, name=f"xbp{i}{b}") for b in range(2)]
                for i in range(IPC)
            ]
            for i in range(IPC):
                for b in range(2):
                    t = xbp[i][b]
                    nc.vector.memset(t[:, 0:67], 0.0)
                    edge = t[:, 65 : 65 + 65 * RL].rearrange(
                        "p (r c) -> p r c", c=RL
                    )
                    nc.vector.memset(edge[:, :, 0:2], 0.0)
                    nc.vector.memset(t[:, 65 * RL : XBPLEN], 0.0)

            pooled = [
                cpool.tile([C, HP, WP], F32, name=f"pooled{i}") for i in range(IPC)
            ]

            for i in range(IPC):
                # ---------- avg-pool (sum of 4; 0.25 folded later) ----------
                xv = x_d[i].rearrange("c (h two) w -> c h two w", two=2)
                for k in range(NCH):
                    rows = slice(POOLCH * k, POOLCH * (k + 1))
                    if os.environ.get("KVAR") == "noaccum":
                        xr = xpool.tile([C, 2 * POOLCH, W], F32, tag="xs",
                                        name=f"xr{i}_{k}")
                        nc.sync.dma_start(
                            xr[:],
                            x_d[i][:, 2 * POOLCH * k : 2 * POOLCH * (k + 1), :],
                        )
                        xs = xpool.tile([C, POOLCH, W], F32, tag="xs2",
                                        name=f"xs{i}_{k}")
                        xrr = xr[:].rearrange("p (h two) w -> p h two w", two=2)
                        nc.vector.tensor_tensor(
                            xs[:], xrr[:, :, 0, :], xrr[:, :, 1, :], ALU.add
                        )
                    else:
                        xs = xpool.tile([C, POOLCH, W], F32, tag="xs",
                                        name=f"xs{i}_{k}")
                        # even rows via HWDGE, odd rows accumulated in the DMA
                        # datapath (SWDGE CCE add)
                        nc.sync.dma_start(xs[:], xv[:, rows, 0, :])
                        nc.gpsimd.dma_start(
                            xs[:], xv[:, rows, 1, :], accum_op=ALU.add
                        )
                    # adjacent-column pairs
                    xsw = xs[:].rearrange("p h (w two) -> p h w two", two=2)
                    eng = nc.vector if k % 2 == 0 else nc.gpsimd
                    eng.tensor_tensor(
                        pooled[i][:, rows, :],
                        xsw[:, :, :, 0],
                        xsw[:, :, :, 1],
                        ALU.add,
                    )
                    # ---------- binary activation: sign(mvk*pool + mvb) -----
                    for b in range(2):
                        xb3 = xbp[i][b][:, :NPADF].rearrange(
                            "p (r c) -> p r c", c=RL
                        )
                        nc.scalar.activation(
                            xb3[:, 1 + POOLCH * k : 1 + POOLCH * (k + 1),
                                1 : 1 + WP],
                            pooled[i][:, rows, :],
                            AF.Sign,
                            bias=pp_t[:, 5 * b + 1 : 5 * b + 2],
                            scale=sc_sign[b][:],
                        )

                # residual-plus-pb1 tiles (also the output staging buffers)
                rss = []
                for b in range(2):
                    rs = respool.tile(
                        [C, HP, WP], F32, tag="res", name=f"res{i}{b}"
                    )
                    eng = nc.vector if b == 0 else nc.gpsimd
                    eng.tensor_scalar(
                        rs[:], pooled[i][:], 0.25,
                        pp_t[:, 5 * b + 4 : 5 * b + 5], ALU.mult, ALU.add,
                    )
                    rss.append(rs)

                # ---------- conv: chunk-major, branches interleaved ----------
                _oadt = F32 if os.environ.get("KVAR") == "f32epi" else FP16
                oas = [
                    opool.tile([C, HP, WP], _oadt, tag="oasm", name=f"oa{i}{b}")
                    for b in range(2)
                ]
                _pairs = (
                    [(ci, b) for b in range(2) for ci in range(len(CHUNKS))]
                    if os.environ.get("KVAR") == "seqbr"
                    else [(ci, b) for ci in range(len(CHUNKS)) for b in range(2)]
                )
                for ci, b in _pairs:
                    (q0, nrows, ncols) = CHUNKS[ci]
                    if True:
                        pt = pspool.tile(
                            [C, NROWCH * RL], F32, tag="ps",
                            name=f"ps{i}{b}{ci}",
                        )
                        for t in range(9):
                            off = (t // 3) * RL + (t % 3)
                            nc.tensor.matmul(
                                pt[:, :ncols],
                                sgn[b][:, C * t : C * (t + 1)],
                                xbp[i][b][:, q0 + off : q0 + off + ncols],
                                start=(t == 0),
                                stop=(t == 8),
                            )
                        # evict valid columns with fused affine
                        # a = scale_w * S + pb0 (bf16 staging)
                        r0 = q0 // RL
                        nc.scalar.activation(
                            oas[b][:, r0 : r0 + nrows, :],
                            pt[:, :ncols].rearrange("p (r c) -> p r c", c=RL)[
                                :, :, :WP
                            ],
                            AF.Identity,
                            bias=pp_t[:, 5 * b + 2 : 5 * b + 3],
                            scale=scale_w[b][:],
                        )

                # ---------- per-half epilogue ----------
                for b in range(2):
                    for hi, (r0, nr) in enumerate(HALVES):
                        part = oas[b][:, r0 : r0 + nr, :]
                        rpart = rss[b][:, r0 : r0 + nr, :]
                        # prelu in place (fp16, 2x DVE mode)
                        nc.vector.scalar_tensor_tensor(
                            part, part, pp_t[:, 5 * b + 3 : 5 * b + 4], part,
                            ALU.mult, ALU.max,
                        )
                        # out = prelu + (residual + pb1), into the fp32 tile
                        eng = nc.gpsimd if (b + hi) % 2 else nc.vector
                        if os.environ.get("KVAR") == "noinplace":
                            fo = opool.tile([C, nr, WP], F32, tag="fo",
                                            name=f"fo{i}{b}{hi}", bufs=2)
                            eng.tensor_tensor(fo[:], part, rpart, ALU.add)
                            nc.scalar.dma_start(
                                out_d[i, C * b : C * (b + 1), r0 : r0 + nr, :],
                                fo[:],
                            )
                        else:
                            eng.tensor_tensor(rpart, part, rpart, ALU.add)
                            nc.scalar.dma_start(
                                out_d[i, C * b : C * (b + 1), r0 : r0 + nr, :],
                                rpart,
                            )

    nc.compile()
    return nc


def _prep_weights(Wb):
    Wb = np.asarray(Wb, dtype=np.float32)
    wn = Wb.reshape(C, C * 9)
    wt = np.ascontiguousarray(
        Wb.reshape(C, C, 9).transpose(1, 2, 0).reshape(C, 9 * C)
    )
    return wt, wn


def _prep_inputs(inputs):
    x = np.ascontiguousarray(np.asarray(inputs["x"], dtype=np.float32))
    wt1, wn1 = _prep_weights(inputs["W1"])
    wt2, wn2 = _prep_weights(inputs["W2"])
    wt = np.ascontiguousarray(np.stack([wt1, wt2]))
    wn = np.ascontiguousarray(np.stack([wn1, wn2]))

    def col(v):
        return np.asarray(v, dtype=np.float32).reshape(C)

    pp = np.zeros((C, 10), dtype=np.float32)
    for b, sfx in enumerate(("1", "2")):
        pp[:, 5 * b + 0] = col(inputs["mvk" + sfx])
        pp[:, 5 * b + 1] = col(inputs["mvb" + sfx])
        pp[:, 5 * b + 2] = col(inputs["pb0_" + sfx])
        pp[:, 5 * b + 3] = col(inputs["alpha" + sfx])
        pp[:, 5 * b + 4] = col(inputs["pb1_" + sfx])
        a = pp[:, 5 * b + 3]
        assert np.all((a >= 0.0) & (a <= 1.0)), (
            "prelu max-identity requires alpha in [0,1]"
        )

    in_maps = [
        {"x": np.ascontiguousarray(x[IPC * c : IPC * (c + 1)]),
         "wt": wt, "wn": wn, "pp": pp}
        for c in range(NCORES)
    ]
    return in_maps


_NC_CACHE = {}


def get_nc():
    if "nc" not in _NC_CACHE:
        _NC_CACHE["nc"] = build_nc()
    return _NC_CACHE["nc"]


def kernel(__trace__=False, **inputs):
    nc = get_nc()
    in_maps = _prep_inputs(inputs)
    res = run_bass_kernel_spmd(
        nc, in_maps, list(range(NCORES)), trace=bool(__trace__)
    )
    out = np.concatenate([res.results[c]["out"] for c in range(NCORES)], axis=0)
    out = np.ascontiguousarray(out.astype(np.float32))
    if __trace__:
        return out, res
    return out


# revision 10
# speedup vs baseline: 1.0913x; 1.0913x over previous
"""Trainium2 Bass kernel for nn_BiSRConv2d_Down.

Reference semantics (forward values):
  out  = avgpool2x2(x)                                  [B, C, H/2, W/2]
  for branch b in {1, 2}:
    xb   = sign(out * mvk_b + mvb_b)                    (tanh STE terms cancel)
    bw   = mean|W_b|_(i,kh,kw) * sign(W_b)              per-output-channel scale
    conv = conv2d(xb, bw, pad=1)
    y_b  = out + (prelu(conv + pb0_b; alpha_b) + pb1_b)
  return concat([y1, y2], channel axis)

Strategy: data-parallel over batch on 8 cores (2 images/core).
 - avg-pool as two tensor-tensor adds (row pairs with contiguous reads, then
   column pairs); the 0.25 scale is folded into the sign activation's scale
   and the residual op.
 - conv = 9 shifted 128x128 matmuls (channels on partitions), chunk-major
   (9 taps back-to-back into one PSUM bank), branches interleaved per chunk.
   sign activations are exact in bf16 so matmuls accumulate exact integer
   sums; the per-output-channel scale rides the PSUM eviction (ScalarE
   activation with per-partition scale/bias).
 - PReLU uses prelu(v) = max(v, alpha*v) (valid for alpha in [0,1]; checked
   on host), computed in fp16 (conv term is ~2% of output
   magnitude and values are small, so fp16 rounding is ~1e-5 relative to the
   output scale; the residual path stays fp32). Final add lands in the fp32 residual tile.
"""

import numpy as np

import concourse.bacc as bacc
import concourse.mybir as mybir
import concourse.tile as tile
from concourse.bass_utils import run_bass_kernel_spmd

F32 = mybir.dt.float32
BF16 = mybir.dt.bfloat16
FP16 = mybir.dt.float16
AF = mybir.ActivationFunctionType
ALU = mybir.AluOpType

B, C, H, W = 16, 128, 128, 128
NCORES = 8
IPC = B // NCORES          # images per core
HP, WP = H // 2, W // 2    # pooled height/width: 64, 64
RL = WP + 2                # padded row stride 66
NPADF = (HP + 2) * RL      # padded image size 4356
XBPLEN = NPADF + 2         # +2 tail pad so tap reads stay in-bounds
NROWCH = 7                 # output rows per PSUM chunk (7*66=462 <= 512)
# (q0, nrows, ncols) per PSUM chunk; the tail chunk is the last single row
CHUNKS = [(ci * NROWCH * RL, NROWCH, NROWCH * RL) for ci in range(9)]
CHUNKS.append((63 * RL, 1, RL))
# epilogue halves: output row ranges
HALVES = [(0, 35), (35, 29)]
POOLCH = 16                # pooled rows per input chunk (32 x rows, ~2.1 MiB)
NCH = HP // POOLCH         # input/pool/sign chunks per image


def build_nc():
    nc = bacc.Bacc(
        "TRN2", target_bir_lowering=False, debug=False, num_devices=NCORES
    )
    x_d = nc.dram_tensor("x", [IPC, C, H, W], F32, kind="ExternalInput")
    # wt: host-relaid weights, wt[b][i, t*128+o] = W_b[o, i, kh, kw], t=kh*3+kw
    wt_d = nc.dram_tensor("wt", [2, C, 9 * C], F32, kind="ExternalInput")
    # wn: natural weights flattened per output channel (for mean|W|)
    wn_d = nc.dram_tensor("wn", [2, C, 9 * C], F32, kind="ExternalInput")
    # pp: per-channel params, col 5*b+{0:mvk,1:mvb,2:pb0,3:alpha,4:pb1}
    pp_d = nc.dram_tensor("pp", [C, 10], F32, kind="ExternalInput")
    out_d = nc.dram_tensor("out", [IPC, 2 * C, HP, WP], F32, kind="ExternalOutput")

    with tile.TileContext(nc) as tc:
        with (
            tc.tile_pool(name="const", bufs=1) as cpool,
            tc.tile_pool(name="wload", bufs=1) as wpool,
            tc.tile_pool(name="xin", bufs=3) as xpool,
            tc.tile_pool(name="oasm", bufs=2) as opool,
            tc.tile_pool(name="resp", bufs=2) as respool,
            tc.tile_pool(name="ps", bufs=6, space="PSUM") as pspool,
        ):
            # ---------- weight / param prep (scalar HWDGE queue, so the
            # sync queue starts streaming x immediately) ----------
            pp_t = cpool.tile([C, 10], F32, name="pp_t")
            nc.scalar.dma_start(pp_t[:], pp_d[:])

            sgn, scale_w, sc_sign = [], [], []
            for b in range(2):
                wld = wpool.tile([C, 9 * C], F32, tag="wload", name=f"wld{b}")
                nc.scalar.dma_start(wld[:], wt_d[b])
                s = cpool.tile([C, 9 * C], BF16, name=f"sgnw{b}")
                nc.scalar.sign(s[:], wld[:])
                sgn.append(s)

                wnl = wpool.tile([C, 9 * C], F32, tag="wload", name=f"wnl{b}")
                nc.scalar.dma_start(wnl[:], wn_d[b])
                asum = cpool.tile([C, 1], F32, name=f"asum{b}")
                nc.scalar.activation(wnl[:], wnl[:], AF.Abs, accum_out=asum[:])
                sw = cpool.tile([C, 1], F32, name=f"scale_w{b}")
                nc.vector.tensor_scalar_mul(sw[:], asum[:], 1.0 / (9 * C))
                scale_w.append(sw)

                ss = cpool.tile([C, 1], F32, name=f"sc_sign{b}")
                nc.vector.tensor_scalar_mul(
                    ss[:], pp_t[:, 5 * b + 0 : 5 * b + 1], 0.25
                )
                sc_sign.append(ss)

            # padded sign-activation buffers: only the BORDERS need zeroing
            # (row 0, row 65, cols 0/65 of each row, 2-elem tail); interiors
            # are fully rewritten per image.
            xbp = [
                [cpool.tile([C, XBPLEN], BF16, name=f"xbp{i}{b}") for b in range(2)]
                for i in range(IPC)
            ]
            for i in range(IPC):
                for b in range(2):
                    t = xbp[i][b]
                    nc.vector.memset(t[:, 0:67], 0.0)
                    edge = t[:, 65 : 65 + 65 * RL].rearrange(
                        "p (r c) -> p r c", c=RL
                    )
                    nc.vector.memset(edge[:, :, 0:2], 0.0)
                    nc.vector.memset(t[:, 65 * RL : XBPLEN], 0.0)

            pooled = [
                cpool.tile([C, HP, WP], F32, name=f"pooled{i}") for i in range(IPC)
            ]

            for i in range(IPC):
                # ---------- avg-pool (sum of 4; 0.25 folded later) ----------
                for k in range(NCH):
                    rows = slice(POOLCH * k, POOLCH * (k + 1))
                    xr = xpool.tile([C, 2 * POOLCH, W], F32, tag="xr",
                                    name=f"xr{i}_{k}")
                    nc.sync.dma_start(
                        xr[:],
                        x_d[i][:, 2 * POOLCH * k : 2 * POOLCH * (k + 1), :],
                    )
                    # row pairs first: contiguous innermost reads
                    xs = xpool.tile([C, POOLCH, W], F32, tag="xs",
                                    name=f"xs{i}_{k}")
                    xrr = xr[:].rearrange("p (h two) w -> p h two w", two=2)
                    nc.vector.tensor_tensor(
                        xs[:], xrr[:, :, 0, :], xrr[:, :, 1, :], ALU.add
                    )
                    # adjacent-column pairs
                    xsw = xs[:].rearrange("p h (w two) -> p h w two", two=2)
                    eng = nc.vector if k % 2 == 0 else nc.gpsimd
                    eng.tensor_tensor(
                        pooled[i][:, rows, :],
                        xsw[:, :, :, 0],
                        xsw[:, :, :, 1],
                        ALU.add,
                    )
                    # ---------- binary activation: sign(mvk*pool + mvb) -----
                    for b in range(2):
                        xb3 = xbp[i][b][:, :NPADF].rearrange(
                            "p (r c) -> p r c", c=RL
                        )
                        nc.scalar.activation(
                            xb3[:, 1 + POOLCH * k : 1 + POOLCH * (k + 1),
                                1 : 1 + WP],
                            pooled[i][:, rows, :],
                            AF.Sign,
                            bias=pp_t[:, 5 * b + 1 : 5 * b + 2],
                            scale=sc_sign[b][:],
                        )

                # residual-plus-pb1 tiles (also the output staging buffers)
                rss = []
                for b in range(2):
                    rs = respool.tile(
                        [C, HP, WP], F32, tag="res", name=f"res{i}{b}"
                    )
                    eng = nc.vector if b == 0 else nc.gpsimd
                    eng.tensor_scalar(
                        rs[:], pooled[i][:], 0.25,
                        pp_t[:, 5 * b + 4 : 5 * b + 5], ALU.mult, ALU.add,
                    )
                    rss.append(rs)

                # ---------- conv: chunk-major, branches interleaved ----------
                oas = [
                    opool.tile([C, HP, WP], FP16, tag="oasm", name=f"oa{i}{b}")
                    for b in range(2)
                ]
                for ci, (q0, nrows, ncols) in enumerate(CHUNKS):
                    for b in range(2):
                        pt = pspool.tile(
                            [C, NROWCH * RL], F32, tag="ps",
                            name=f"ps{i}{b}{ci}",
                        )
                        for t in range(9):
                            off = (t // 3) * RL + (t % 3)
                            nc.tensor.matmul(
                                pt[:, :ncols],
                                sgn[b][:, C * t : C * (t + 1)],
                                xbp[i][b][:, q0 + off : q0 + off + ncols],
                                start=(t == 0),
                                stop=(t == 8),
                            )
                        # evict valid columns with fused affine
                        # a = scale_w * S + pb0 (bf16 staging)
                        r0 = q0 // RL
                        nc.scalar.activation(
                            oas[b][:, r0 : r0 + nrows, :],
                            pt[:, :ncols].rearrange("p (r c) -> p r c", c=RL)[
                                :, :, :WP
                            ],
                            AF.Identity,
                            bias=pp_t[:, 5 * b + 2 : 5 * b + 3],
                            scale=scale_w[b][:],
                        )

                # ---------- per-half epilogue ----------
                for b in range(2):
                    for hi, (r0, nr) in enumerate(HALVES):
                        part = oas[b][:, r0 : r0 + nr, :]
                        rpart = rss[b][:, r0 : r0 + nr, :]
                        # prelu in place (fp16, 2x DVE mode)
                        nc.vector.scalar_tensor_tensor(
                            part, part, pp_t[:, 5 * b + 3 : 5 * b + 4], part,
                            ALU.mult, ALU.max,
                        )
                        # out = prelu + (residual + pb1), into the fp32 tile
                        eng = nc.gpsimd if (b + hi) % 2 else nc.vector
                        eng.tensor_tensor(rpart, part, rpart, ALU.add)
                        nc.sync.dma_start(
                            out_d[i, C * b : C * (b + 1), r0 : r0 + nr, :],
                            rpart,
                        )

    nc.compile()
    return nc


def _prep_weights(Wb):
    Wb = np.asarray(Wb, dtype=np.float32)
    wn = Wb.reshape(C, C * 9)
    wt = np.ascontiguousarray(
        Wb.reshape(C, C, 9).transpose(1, 2, 0).reshape(C, 9 * C)
    )
    return wt, wn


def _prep_inputs(inputs):
    x = np.ascontiguousarray(np.asarray(inputs["x"], dtype=np.float32))
    wt1, wn1 = _prep_weights(inputs["W1"])
    wt2, wn2 = _prep_weights(inputs["W2"])
    wt = np.ascontiguousarray(np.stack([wt1, wt2]))
    wn = np.ascontiguousarray(np.stack([wn1, wn2]))

    def col(v):
        return np.asarray(v, dtype=np.float32).reshape(C)

    pp = np.zeros((C, 10), dtype=np.float32)
    for b, sfx in enumerate(("1", "2")):
        pp[:, 5 * b + 0] = col(inputs["mvk" + sfx])
        pp[:, 5 * b + 1] = col(inputs["mvb" + sfx])
        pp[:, 5 * b + 2] = col(inputs["pb0_" + sfx])
        pp[:, 5 * b + 3] = col(inputs["alpha" + sfx])
        pp[:, 5 * b + 4] = col(inputs["pb1_" + sfx])
        a = pp[:, 5 * b + 3]
        assert np.all((a >= 0.0) & (a <= 1.0)), (
            "prelu max-identity requires alpha in [0,1]"
        )

    in_maps = [
        {"x": np.ascontiguousarray(x[IPC * c : IPC * (c + 1)]),
         "wt": wt, "wn": wn, "pp": pp}
        for c in range(NCORES)
    ]
    return in_maps


_NC_CACHE = {}


def get_nc():
    if "nc" not in _NC_CACHE:
        _NC_CACHE["nc"] = build_nc()
    return _NC_CACHE["nc"]


def kernel(__trace__=False, **inputs):
    nc = get_nc()
    in_maps = _prep_inputs(inputs)
    res = run_bass_kernel_spmd(
        nc, in_maps, list(range(NCORES)), trace=bool(__trace__)
    )
    out = np.concatenate([res.results[c]["out"] for c in range(NCORES)], axis=0)
    out = np.ascontiguousarray(out.astype(np.float32))
    if __trace__:
        return out, res
    return out


# revision 12
# speedup vs baseline: 1.2469x; 1.1426x over previous
"""Trainium2 Bass kernel for nn_BiSRConv2d_Down.

Reference semantics (forward values):
  out  = avgpool2x2(x)                                  [B, C, H/2, W/2]
  for branch b in {1, 2}:
    xb   = sign(out * mvk_b + mvb_b)                    (tanh STE terms cancel)
    bw   = mean|W_b|_(i,kh,kw) * sign(W_b)              per-output-channel scale
    conv = conv2d(xb, bw, pad=1)
    y_b  = out + (prelu(conv + pb0_b; alpha_b) + pb1_b)
  return concat([y1, y2], channel axis)

Strategy: data-parallel over batch on 8 cores (2 images/core).
 - avg-pool as two tensor-tensor adds (row pairs with contiguous reads, then
   column pairs); the 0.25 scale is folded into the sign activation's scale
   and the final residual op.
 - conv = 9 shifted 128x128 matmuls (channels on partitions), chunk-major
   (9 taps back-to-back into one PSUM bank), branches interleaved per chunk.
   sign activations are exact in bf16 so matmuls accumulate exact integer
   sums; the per-output-channel scale and bias ride the PSUM eviction
   (ScalarE activation with per-partition scale/bias).
 - epilogue: prelu(v)+pb1 = max(v+pb1, alpha*v+pb1) for alpha in [0,1]
   (checked on host). The eviction writes a1 = v+pb1 in fp16, the second arm
   is a2 = alpha*a1 + (1-alpha)*pb1 (fp16 tensor_scalar), then max(a1,a2)
   and one fp32 scalar_tensor_tensor adds the 0.25-scaled pooled residual.
   The conv term is ~2% of the output magnitude so fp16 staging contributes
   ~1e-5 relative error; the residual path stays fp32.
"""

import numpy as np

import concourse.bacc as bacc
import concourse.mybir as mybir
import concourse.tile as tile
from concourse.bass_utils import run_bass_kernel_spmd

F32 = mybir.dt.float32
BF16 = mybir.dt.bfloat16
FP16 = mybir.dt.float16
AF = mybir.ActivationFunctionType
ALU = mybir.AluOpType

B, C, H, W = 16, 128, 128, 128
NCORES = 8
IPC = B // NCORES          # images per core
HP, WP = H // 2, W // 2    # pooled height/width: 64, 64
RL = WP + 2                # padded row stride 66
NPADF = (HP + 2) * RL      # padded image size 4356
XBPLEN = NPADF + 2         # +2 tail pad so tap reads stay in-bounds
NROWCH = 7                 # output rows per PSUM chunk (7*66=462 <= 512)
# (q0, nrows, ncols) per PSUM chunk; the tail chunk is the last single row
CHUNKS = [(ci * NROWCH * RL, NROWCH, NROWCH * RL) for ci in range(9)]
CHUNKS.append((63 * RL, 1, RL))
# epilogue halves: output row ranges
HALVES = [(0, 35), (35, 29)]
# pooled-row counts per input chunk: small first chunks let the first sign
# tiles (and therefore the first matmuls) start early
POOLCHS = [4, 4, 8, 16, 16, 16]


def build_nc():
    nc = bacc.Bacc(
        "TRN2", target_bir_lowering=False, debug=False, num_devices=NCORES
    )
    x_d = nc.dram_tensor("x", [IPC, C, H, W], F32, kind="ExternalInput")
    # wt: host-relaid weights, wt[b][i, t*128+o] = W_b[o, i, kh, kw], t=kh*3+kw
    wt_d = nc.dram_tensor("wt", [2, C, 9 * C], F32, kind="ExternalInput")
    # wn: natural weights flattened per output channel (for mean|W|)
    wn_d = nc.dram_tensor("wn", [2, C, 9 * C], F32, kind="ExternalInput")
    # pp: per-channel params, col 5*b+{0:mvk,1:mvb,2:pb0,3:alpha,4:pb1}
    pp_d = nc.dram_tensor("pp", [C, 10], F32, kind="ExternalInput")
    out_d = nc.dram_tensor("out", [IPC, 2 * C, HP, WP], F32, kind="ExternalOutput")

    with tile.TileContext(nc) as tc:
        with (
            tc.tile_pool(name="const", bufs=1) as cpool,
            tc.tile_pool(name="wload", bufs=2) as wpool,
            tc.tile_pool(name="xin", bufs=2) as xpool,
            tc.tile_pool(name="xsum", bufs=2) as xspool,
            tc.tile_pool(name="oasm", bufs=2) as opool,
            tc.tile_pool(name="a2p", bufs=2) as a2pool,
            tc.tile_pool(name="outp", bufs=2) as outpool,
            tc.tile_pool(name="ps", bufs=6, space="PSUM") as pspool,
        ):
            # ---------- params + branch-0 sign-weights first (they gate the
            # very first matmul; wn/mean|W| prep is deferred) ----------
            pp_t = cpool.tile([C, 10], F32, name="pp_t")
            nc.sync.dma_start(pp_t[:], pp_d[:])

            sgn = [cpool.tile([C, 9 * C], BF16, name=f"sgnw{b}") for b in range(2)]
            wld = [
                wpool.tile([C, 9 * C], F32, tag="wload", name=f"wld{b}")
                for b in range(2)
            ]
            nc.sync.dma_start(wld[0][:], wt_d[0])
            nc.scalar.sign(sgn[0][:], wld[0][:])

            sc_sign = []
            for b in range(2):
                ss = cpool.tile([C, 1], F32, name=f"sc_sign{b}")
                nc.vector.tensor_scalar_mul(
                    ss[:], pp_t[:, 5 * b + 0 : 5 * b + 1], 0.25
                )
                sc_sign.append(ss)

            # padded sign-activation buffers: only the BORDERS need zeroing
            # (row 0, row 65, cols 0/65 of each row, 2-elem tail); interiors
            # are fully rewritten per image.
            xbp = [
                [cpool.tile([C, XBPLEN], BF16, name=f"xbp{i}{b}") for b in range(2)]
                for i in range(IPC)
            ]
            for i in range(IPC):
                for b in range(2):
                    t = xbp[i][b]
                    nc.vector.memset(t[:, 0:67], 0.0)
                    edge = t[:, 65 : 65 + 65 * RL].rearrange(
                        "p (r c) -> p r c", c=RL
                    )
                    nc.vector.memset(edge[:, :, 0:2], 0.0)
                    nc.vector.memset(t[:, 65 * RL : XBPLEN], 0.0)

            pooled = [
                cpool.tile([C, HP, WP], F32, name=f"pooled{i}") for i in range(IPC)
            ]

            def pool_and_sign(i, k, r0, pch, eng2):
                """DMA 2*pch x rows, pool into pooled[i][r0:r0+pch], and
                write both branches' sign tiles."""
                rows = slice(r0, r0 + pch)
                xr = xpool.tile([C, 32, W], F32, tag="xr", name=f"xr{i}_{k}")
                nc.sync.dma_start(
                    xr[:, : 2 * pch, :], x_d[i][:, 2 * r0 : 2 * (r0 + pch), :]
                )
                xs = xspool.tile([C, 16, W], F32, tag="xs", name=f"xs{i}_{k}")
                xrr = xr[:, : 2 * pch, :].rearrange(
                    "p (h two) w -> p h two w", two=2
                )
                eng2.tensor_tensor(
                    xs[:, :pch, :], xrr[:, :, 0, :], xrr[:, :, 1, :], ALU.add
                )
                xsw = xs[:, :pch, :].rearrange("p h (w two) -> p h w two", two=2)
                nc.gpsimd.tensor_tensor(
                    pooled[i][:, rows, :], xsw[:, :, :, 0], xsw[:, :, :, 1],
                    ALU.add,
                )
                for b in range(2):
                    xb3 = xbp[i][b][:, :NPADF].rearrange("p (r c) -> p r c", c=RL)
                    nc.scalar.activation(
                        xb3[:, 1 + r0 : 1 + r0 + pch, 1 : 1 + WP],
                        pooled[i][:, rows, :],
                        AF.Sign,
                        bias=pp_t[:, 5 * b + 1 : 5 * b + 2],
                        scale=sc_sign[b][:],
                    )

            # first two pool chunks of image 0 interleave with the remaining
            # weight prep, so neither gates the other on DMA/ACT queues
            pool_and_sign(0, 0, 0, POOLCHS[0], nc.vector)

            nc.sync.dma_start(wld[1][:], wt_d[1])
            nc.scalar.sign(sgn[1][:], wld[1][:])

            pool_and_sign(0, 1, POOLCHS[0], POOLCHS[1], nc.vector)

            # mean|W| scales + derived per-channel constants
            scale_w, c1s, c3s = [], [], []
            wnl = [
                wpool.tile([C, 9 * C], F32, tag="wload", name=f"wnl{b}")
                for b in range(2)
            ]
            for b in range(2):
                nc.sync.dma_start(wnl[b][:], wn_d[b])
                asum = cpool.tile([C, 1], F32, name=f"asum{b}")
                nc.scalar.activation(wnl[b][:], wnl[b][:], AF.Abs,
                                     accum_out=asum[:])
                sw = cpool.tile([C, 1], F32, name=f"scale_w{b}")
                nc.vector.tensor_scalar_mul(sw[:], asum[:], 1.0 / (9 * C))
                scale_w.append(sw)
                # c1 = pb0 + pb1 (eviction bias), c3 = (1 - alpha) * pb1
                c1 = cpool.tile([C, 1], F32, name=f"c1_{b}")
                nc.vector.tensor_tensor(
                    c1[:], pp_t[:, 5 * b + 2 : 5 * b + 3],
                    pp_t[:, 5 * b + 4 : 5 * b + 5], ALU.add,
                )
                c1s.append(c1)
                apb1 = cpool.tile([C, 1], F32, name=f"apb1_{b}")
                nc.vector.tensor_tensor(
                    apb1[:], pp_t[:, 5 * b + 3 : 5 * b + 4],
                    pp_t[:, 5 * b + 4 : 5 * b + 5], ALU.mult,
                )
                c3 = cpool.tile([C, 1], F32, name=f"c3_{b}")
                nc.vector.tensor_tensor(
                    c3[:], pp_t[:, 5 * b + 4 : 5 * b + 5], apb1[:], ALU.subtract,
                )
                c3s.append(c3)

            for i in range(IPC):
                # remaining pool chunks for this image
                start_k = 2 if i == 0 else 0
                r0 = sum(POOLCHS[:start_k]) if i == 0 else 0
                for k in range(start_k, len(POOLCHS)):
                    eng2 = nc.gpsimd if k == 3 else nc.vector
                    pool_and_sign(i, k, r0, POOLCHS[k], eng2)
                    r0 += POOLCHS[k]

                # ---------- conv: chunk-major, branches interleaved ----------
                oas = [
                    opool.tile([C, HP, WP], FP16, tag="oasm", name=f"oa{i}{b}")
                    for b in range(2)
                ]
                for ci, (q0, nrows, ncols) in enumerate(CHUNKS):
                    for b in range(2):
                        pt = pspool.tile(
                            [C, NROWCH * RL], F32, tag="ps",
                            name=f"ps{i}{b}{ci}",
                        )
                        for t in range(9):
                            off = (t // 3) * RL + (t % 3)
                            nc.tensor.matmul(
                                pt[:, :ncols],
                                sgn[b][:, C * t : C * (t + 1)],
                                xbp[i][b][:, q0 + off : q0 + off + ncols],
                                start=(t == 0),
                                stop=(t == 8),
                            )
                        # evict valid columns: a1 = scale_w*S + (pb0 + pb1)
                        cr = q0 // RL
                        nc.scalar.activation(
                            oas[b][:, cr : cr + nrows, :],
                            pt[:, :ncols].rearrange("p (r c) -> p r c", c=RL)[
                                :, :, :WP
                            ],
                            AF.Identity,
                            bias=c1s[b][:],
                            scale=scale_w[b][:],
                        )

                # ---------- per-half epilogue ----------
                for b in range(2):
                    fo = outpool.tile([C, HP, WP], F32, tag="fo",
                                      name=f"fo{i}{b}")
                    for hi, (hr0, nr) in enumerate(HALVES):
                        a1 = oas[b][:, hr0 : hr0 + nr, :]
                        a2f = a2pool.tile([C, 35, WP], FP16, tag="a2",
                                          name=f"a2_{i}{b}{hi}")
                        a2 = a2f[:, :nr, :]
                        # a2 = alpha*a1 + (1-alpha)*pb1
                        nc.vector.tensor_scalar(
                            a2, a1, pp_t[:, 5 * b + 3 : 5 * b + 4], c3s[b][:],
                            ALU.mult, ALU.add,
                        )
                        # a1 = max(a1, a2) = prelu(v) + pb1 (fp16 is
                        # DVE-only; 2x mode there)
                        nc.vector.tensor_tensor(a1, a1, a2, ALU.max)
                        # out = 0.25*pooled + (prelu + pb1)
                        nc.vector.scalar_tensor_tensor(
                            fo[:, hr0 : hr0 + nr, :],
                            pooled[i][:, hr0 : hr0 + nr, :], 0.25, a1,
                            ALU.mult, ALU.add,
                        )
                        nc.sync.dma_start(
                            out_d[i, C * b : C * (b + 1), hr0 : hr0 + nr, :],
                            fo[:, hr0 : hr0 + nr, :],
                        )

    nc.compile()
    return nc


def _prep_weights(Wb):
    Wb = np.asarray(Wb, dtype=np.float32)
    wn = Wb.reshape(C, C * 9)
    wt = np.ascontiguousarray(
        Wb.reshape(C, C, 9).transpose(1, 2, 0).reshape(C, 9 * C)
    )
    return wt, wn


def _prep_inputs(inputs):
    x = np.ascontiguousarray(np.asarray(inputs["x"], dtype=np.float32))
    wt1, wn1 = _prep_weights(inputs["W1"])
    wt2, wn2 = _prep_weights(inputs["W2"])
    wt = np.ascontiguousarray(np.stack([wt1, wt2]))
    wn = np.ascontiguousarray(np.stack([wn1, wn2]))

    def col(v):
        return np.asarray(v, dtype=np.float32).reshape(C)

    pp = np.zeros((C, 10), dtype=np.float32)
    for b, sfx in enumerate(("1", "2")):
        pp[:, 5 * b + 0] = col(inputs["mvk" + sfx])
        pp[:, 5 * b + 1] = col(inputs["mvb" + sfx])
        pp[:, 5 * b + 2] = col(inputs["pb0_" + sfx])
        pp[:, 5 * b + 3] = col(inputs["alpha" + sfx])
        pp[:, 5 * b + 4] = col(inputs["pb1_" + sfx])
        a = pp[:, 5 * b + 3]
        assert np.all((a >= 0.0) & (a <= 1.0)), (
            "prelu max-identity requires alpha in [0,1]"
        )

    in_maps = [
        {"x": np.ascontiguousarray(x[IPC * c : IPC * (c + 1)]),
         "wt": wt, "wn": wn, "pp": pp}
        for c in range(NCORES)
    ]
    return in_maps


_NC_CACHE = {}


def get_nc():
    if "nc" not in _NC_CACHE:
        _NC_CACHE["nc"] = build_nc()
    return _NC_CACHE["nc"]


def kernel(__trace__=False, **inputs):
    nc = get_nc()
    in_maps = _prep_inputs(inputs)
    res = run_bass_kernel_spmd(
        nc, in_maps, list(range(NCORES)), trace=bool(__trace__)
    )
    out = np.concatenate([res.results[c]["out"] for c in range(NCORES)], axis=0)
    out = np.ascontiguousarray(out.astype(np.float32))
    if __trace__:
        return out, res
    return out
